# revision 20
# baseline (speedup 1.0000x reference)
"""TRN2 Bass kernel for nn_Actor (moe_routing): gated mixture actor forward.

Strategy: pure data parallel over 8 NeuronCores (batch 65536 -> 8192/core),
weights replicated. On-device layout keeps features in partitions and batch
in the free dimension, so activations stream through the TensorEngine as the
moving operand and weights stay stationary. The host pre-transposes obs and
actions, so the device does zero transposes.

Matmuls run in bf16 (1 cycle/row; fp32 is 4, float32r 1.5) accumulating in
fp32 PSUM. Elementwise ops run on [128, 1024] macro-tiles (two 512-wide
matmul halves per PSUM pair-tile) to amortize the ACT engine's 352-cycle
per-op overhead; leaky-relu work is split across ACT (fused Prelu+bias from
PSUM) and DVE (3-op chain). The per-primitive output heads (20 mu + 20
sigma cols each) are packed four-to-a-PSUM-tile at 32-partition col
positions (tile_position col packing). Biases ride the ACT ops; the mu-head
bias is folded into the primitive reduction as a weighted selection matrix,
and the gate weighting is folded into the sigma exponent
(w*exp(-s) = exp(glogit - s)) via a negated broadcast matmul accumulated
into the same PSUM tile. log_prob/entropy come from one K=64 reduction
matmul whose constant term rides a ln(e)=1 row.

The work is a 6-stage software pipeline across macro-tiles (DMA; L1; L2/L3/
gate-head/towers; tower-out+mixture; reduction+z/ln; final matmul+store),
emitted deepest-stage-first, so every TensorEngine instruction's inputs were
produced at least one full macro-tile earlier and the PE never stalls on the
ACT/DVE chains. All activation functions (Prelu/Exp/Ln) come from one ACT
table set (natural_log_exp_and_others, forced via a table-selection patch)
so there are zero mid-kernel table reloads.
"""

import numpy as np
import ml_dtypes

import concourse.bass as bass
import concourse.tile as tile
from concourse import bacc, mybir
from concourse.bass_utils import run_bass_kernel_spmd

F32 = mybir.dt.float32
F32R = mybir.dt.float32r
BF16 = mybir.dt.bfloat16

B = 65536
N_CORES = 8
BC = B // N_CORES          # rows per core
NT = 512                   # matmul moving width / PSUM bank width
MT = 1024                  # macro-tile batch width (elementwise op width)
NMAC = BC // MT            # 8
NTILES = BC // NT          # 16
A = 20                     # action dim
P = 8                      # primitives
C = 0.5 * float(np.log(2.0 * np.pi))  # LOG_SQRT_2PI
ALPHA = 0.01

# number of tower-h chunks (out of 16) whose leaky relu runs on DVE not ACT
N_H_DVE = 6

_WPK_COLS = 10352
_BIA_COLS = 41


def _force_single_act_table():
    """Strip Exp/Ln/Prelu from every activation-function set except
    natural_log_exp_and_others, so bacc's table-load placement resolves all
    three to one set and hoists a single load (instead of ping-ponging
    between sets every tile, 1.3us per switch)."""
    import concourse.hw_specs as hw_specs
    import concourse.bacc as bacc_mod

    orig = hw_specs.get_activation_tables
    KEEP = "natural_log_exp_and_others"
    STRIP = {
        mybir.ActivationFunctionType.Exp,
        mybir.ActivationFunctionType.Ln,
        mybir.ActivationFunctionType.Prelu,
    }

    def patched(arch):
        tabs = dict(orig(arch))
        out = {}
        for name, fns in tabs.items():
            if name == KEEP:
                out[name] = fns
            else:
                out[name] = {f for f in fns if f not in STRIP}
        return out

    bacc_mod.get_activation_tables = patched


# ---------------------------------------------------------------- host packs
def _chunk_cols(w):
    """[K, M] -> [128, (K/128)*M] chunked lhsT layout."""
    K, M = w.shape
    nk = K // 128
    out = np.empty((128, nk * M), dtype=np.float32)
    for k in range(nk):
        out[:, k * M : (k + 1) * M] = w[k * 128 : (k + 1) * 128]
    return out


def _build_packs(inp):
    wcols = []
    woff = {}

    def addw(name, arr):
        woff[name] = sum(a.shape[1] for a in wcols)
        wcols.append(arr.astype(np.float32))

    addw("W1", _chunk_cols(inp["W1"]))
    addw("Ws1", _chunk_cols(inp["Ws1"]))
    addw("Wg1", _chunk_cols(inp["Wg1"]))
    addw("W2", _chunk_cols(inp["W2"]))
    for p in range(P):
        addw(f"Wp1_{p}", _chunk_cols(inp["Wp1"][p]))
    addw("Ws2", _chunk_cols(inp["Ws2"]))
    addw("Ws3", _chunk_cols(inp["Ws3"]))
    addw("Wg2", _chunk_cols(inp["Wg2"]))
    addw("Wg3", _chunk_cols(inp["Wg3"]))
    addw("Wgate", _chunk_cols(inp["Wgate"]))
    ssel = np.zeros((128, 32), np.float32)
    for j in range(4):
        for a in range(A):
            ssel[32 * j + a, a] = 1.0
    addw("Ssel", ssel)
    # weighted selection: adds sum_p bmu_p * inv_p to sum1 (mu-bias term)
    for g in range(2):
        sb = np.zeros((128, 32), np.float32)
        for j in range(4):
            sb[32 * j : 32 * j + A, :A] = np.diag(inp["bp2"][4 * g + j][:A])
        addw(f"SselB{g}", sb)
    wpk = np.concatenate(wcols, axis=1).astype(ml_dtypes.bfloat16)
    assert wpk.shape[1] == _WPK_COLS, wpk.shape
    finw = np.zeros((128, 2), np.float32)
    finw[0:A, 0] = -0.5
    finw[32 : 32 + A, 0] = 1.0
    finw[32 : 32 + A, 1] = -1.0
    finw[63, 0] = -A * C
    finw[63, 1] = A * (0.5 + C)
    finw = finw.astype(ml_dtypes.bfloat16)

    # bf16 tower-out weights: [128, 8 * 128], order (pass mu/sig, group, kchunk)
    bpk = np.zeros((128, 8, 128), np.float32)
    bi = 0
    boff = {}
    for part, off in (("mu", 0), ("sig", A)):
        for g in range(2):
            for k in range(2):
                boff[(part, g, k)] = bi
                for j in range(4):
                    w = inp["Wp2"][4 * g + j][128 * k : 128 * k + 128, off : off + A]
                    bpk[:, bi, 32 * j : 32 * j + A] = w
                bi += 1
    bpk = bpk.reshape(128, 8 * 128).astype(ml_dtypes.bfloat16)

    # ACT bias pack [128, CA] fp32
    bcols = []
    aoff = {}

    def addb(name, vec):
        aoff[name] = len(bcols)
        col = np.zeros(128, np.float32)
        col[: len(vec)] = vec
        bcols.append(col)

    for m in range(4):
        addb(f"b1_{m}", inp["b1"][128 * m : 128 * m + 128])
    for m in range(2):
        addb(f"b2_{m}", inp["b2"][128 * m : 128 * m + 128])
    for p in range(P):
        for m in range(2):
            addb(f"bp1_{p}_{m}", inp["bp1"][p][128 * m : 128 * m + 128])
    for m in range(4):
        addb(f"bs1_{m}", inp["bs1"][128 * m : 128 * m + 128])
    for m in range(2):
        addb(f"bs2_{m}", inp["bs2"][128 * m : 128 * m + 128])
    addb("bs3_0", inp["bs3"])
    for m in range(4):
        addb(f"bg1_{m}", inp["bg1"][128 * m : 128 * m + 128])
    for m in range(2):
        addb(f"bg2_{m}", inp["bg2"][128 * m : 128 * m + 128])
    addb("bg3_0", inp["bg3"])
    addb("bgate", inp["bgate"])
    lnb = np.zeros(128, np.float32)
    lnb[A:32] = 1.0
    lnb[31] = float(np.e)  # stack row 63 becomes ln(e)=1 -> constant term
    addb("lnbias", lnb)
    addb("zero", np.zeros(1, np.float32))
    for g in range(2):  # negated sigma-logit bias, group layout
        nb = np.zeros(128, np.float32)
        for j in range(4):
            nb[32 * j : 32 * j + A] = -inp["bp2"][4 * g + j][A:]
        addb(f"nbsig_{g}", nb)
    bia = np.stack(bcols, axis=1)
    assert bia.shape[1] == _BIA_COLS, bia.shape

    bsel = np.zeros((8, 256), np.float32)
    for g in range(2):
        for j in range(4):
            bsel[4 * g + j, 128 * g + 32 * j : 128 * g + 32 * j + A] = -1.0
    bsel = bsel.astype(ml_dtypes.bfloat16)

    return wpk, woff, bpk, boff, bia, aoff, bsel, finw


# ---------------------------------------------------------------- bass build
def _build_nc():
    _force_single_act_table()
    nc = bacc.Bacc("TRN2", target_bir_lowering=False, debug=False,
                   num_devices=N_CORES)

    obs_t = nc.dram_tensor("obs_t", [384, BC], BF16, kind="ExternalInput").ap()
    act_t = nc.dram_tensor("act_t", [32, BC], F32, kind="ExternalInput").ap()
    wpk_d = nc.dram_tensor("wpk", [128, _WPK_COLS], BF16, kind="ExternalInput").ap()
    bpk_d = nc.dram_tensor("bpk", [128, 1024], BF16, kind="ExternalInput").ap()
    bia_d = nc.dram_tensor("bia", [128, _BIA_COLS], F32, kind="ExternalInput").ap()
    bsel_d = nc.dram_tensor("bsel", [8, 256], BF16, kind="ExternalInput").ap()
    finw_d = nc.dram_tensor("finw", [128, 2], BF16, kind="ExternalInput").ap()
    out_t = nc.dram_tensor("out_t", [2, BC], F32, kind="ExternalOutput").ap()

    _, woff, _, boff, _, aoff, _, _ = _build_packs_dummy()

    PR = mybir.ActivationFunctionType.Prelu
    EX = mybir.ActivationFunctionType.Exp
    LN = mybir.ActivationFunctionType.Ln

    with tile.TileContext(nc) as tc:
        with (
            tc.tile_pool(name="wgt", bufs=1) as wgt,
            tc.tile_pool(name="obs", bufs=2) as obsp,
            tc.tile_pool(name="quad", bufs=4) as quad,
            tc.tile_pool(name="dual", bufs=3) as dual,
            tc.tile_pool(name="uni", bufs=3) as uni,
            tc.tile_pool(name="mix4", bufs=6) as mix4,
            tc.tile_pool(name="lr", bufs=2) as lrp,
            tc.tile_pool(name="hp", bufs=18) as hp,
            tc.tile_pool(name="smA", bufs=5) as smA,
            tc.tile_pool(name="smB", bufs=3) as smB,
            tc.tile_pool(name="smC", bufs=3) as smC,
            tc.tile_pool(name="psA", bufs=3, space="PSUM") as psA,
            tc.tile_pool(name="psS", bufs=2, space="PSUM") as psS,
        ):
            SPLIT = 2560  # W1 + Ws1 + Wg1: needed by the first L1 matmuls
            wpkA = wgt.tile([128, SPLIT], BF16)
            wpkB = wgt.tile([128, _WPK_COLS - SPLIT], BF16)
            bpk = wgt.tile([128, 1024], BF16)
            bia = wgt.tile([128, _BIA_COLS], F32)
            bsel = wgt.tile([8, 256], BF16)
            finw = wgt.tile([128, 2], BF16)
            nc.sync.dma_start(out=bia, in_=bia_d)
            nc.sync.dma_start(out=wpkA, in_=wpk_d[:, 0:SPLIT])
            nc.sync.dma_start(out=wpkB, in_=wpk_d[:, SPLIT:_WPK_COLS])
            nc.sync.dma_start(out=bpk, in_=bpk_d)
            nc.sync.dma_start(out=bsel, in_=bsel_d)
            nc.sync.dma_start(out=finw, in_=finw_d)

            def W(name, k, M):
                off = woff[name] + k * M
                if off < SPLIT:
                    return wpkA[:, off : off + M]
                return wpkB[:, off - SPLIT : off - SPLIT + M]

            def BW(part, g, k):
                i = boff[(part, g, k)]
                return bpk[:, 128 * i : 128 * i + 128]

            def bias(name):
                return bia[:, aoff[name] : aoff[name] + 1]

            def biasn(name, n):
                return bia[0:n, aoff[name] : aoff[name] + 1]

            def layer(x_chunks, wname, bname, n_in, n_out, pool, out_tag,
                      eng="act", out_dt=BF16):
                engs = eng if isinstance(eng, list) else [eng] * n_out
                outs = []
                for m in range(n_out):
                    eng = engs[m]
                    ps = psA.tile([128, MT], F32, tag="mm")
                    for k in range(n_in):
                        wk = W(wname, k, 128 * n_out)[:, 128 * m : 128 * m + 128]
                        for hh in range(2):
                            nc.tensor.matmul(
                                ps[:, NT * hh : NT * hh + NT], lhsT=wk,
                                rhs=x_chunks[k][:, NT * hh : NT * hh + NT],
                                start=(k == 0), stop=(k == n_in - 1),
                            )
                    o = pool.tile([128, MT], out_dt, tag=out_tag)
                    b = bias(f"{bname}_{m}")
                    if eng == "dve":
                        t = lrp.tile([128, MT], BF16, tag="lrt")
                        u = lrp.tile([128, MT], BF16, tag="lru")
                        nc.vector.tensor_scalar_add(t, ps, b)
                        nc.vector.tensor_scalar_mul(u, t, ALPHA)
                        nc.vector.tensor_max(o, t, u)
                    else:
                        nc.scalar.activation(out=o, in_=ps, func=PR, bias=b,
                                             scale=1.0, alpha=ALPHA)
                    outs.append(o)
                return outs

            def stage0(mt):
                b0 = mt * MT
                obs0 = obsp.tile([128, MT], BF16, tag="o0")
                obs1 = obsp.tile([128, MT], BF16, tag="o1")
                obs2 = obsp.tile([128, MT], BF16, tag="o2")
                nc.sync.dma_start(out=obs0, in_=obs_t[0:128, b0 : b0 + MT])
                nc.sync.dma_start(out=obs1, in_=obs_t[128:256, b0 : b0 + MT])
                nc.sync.dma_start(out=obs2, in_=obs_t[256:384, b0 : b0 + MT])
                aT = smA.tile([32, MT], F32, tag="aT")
                nc.sync.dma_start(out=aT, in_=act_t[:, b0 : b0 + MT])
                return dict(b0=b0, aT=aT, obs=[obs0, obs1, obs2])

            def stage1(st):
                obs0, obs1, obs2 = st["obs"]
                t1 = layer([obs0, obs1], "Ws1", "bs1", 2, 4, quad, "gs1",
                           eng=["act", "dve", "act", "dve"])
                v1 = layer([obs2], "Wg1", "bg1", 1, 4, quad, "gg1",
                           eng=["act", "dve", "act", "act"])
                s1 = layer([obs0, obs1], "W1", "b1", 2, 4, quad, "s1",
                           eng=["act", "dve", "act", "act"])
                return dict(b0=st["b0"], aT=st["aT"], t1=t1, v1=v1, s1=s1)

            def stage2(st):
                t2 = layer(st["t1"], "Ws2", "bs2", 4, 2, dual, "gs2")
                v2 = layer(st["v1"], "Wg2", "bg2", 4, 2, dual, "gg2")
                s1b = layer(st["s1"], "W2", "b2", 4, 2, dual, "s1b")
                s2 = layer(t2, "Ws3", "bs3", 2, 1, uni, "gs3")[0]
                g2 = layer(v2, "Wg3", "bg3", 2, 1, uni, "gg3")[0]
                wg = smB.tile([8, MT], BF16, tag="wg")
                for hh in range(2):
                    off = NT * hh
                    psg = psS.tile([8, NT], F32, tag="small")
                    nc.tensor.matmul(psg, lhsT=W("Wgate", 0, 8),
                                     rhs=s2[:, off : off + NT],
                                     start=True, stop=False)
                    nc.tensor.matmul(psg, lhsT=W("Wgate", 1, 8),
                                     rhs=g2[:, off : off + NT],
                                     start=False, stop=True)
                    nc.scalar.activation(out=wg[:, off : off + NT], in_=psg,
                                         func=mybir.ActivationFunctionType.Identity,
                                         bias=biasn("bgate", 8), scale=1.0)
                ENG = ["dve", "act", "act", "act", "dve", "act", "act", "act"]
                h = []
                for p in range(P):
                    h.append(layer(s1b, f"Wp1_{p}", f"bp1_{p}", 2, 2, hp,
                                   "h", eng=ENG[p], out_dt=BF16))
                return dict(b0=st["b0"], aT=st["aT"], wg=wg, h=h)

            def stage3(st):
                h, wg = st["h"], st["wg"]
                halves = []
                for hh in range(2):
                    off = NT * hh
                    inv, minv = [], []
                    for g in range(2):
                        pg = psA.tile([128, MT], F32, tag="mm")
                        # -glogit broadcast seeds the sig half (start=True)
                        nc.tensor.matmul(pg[:, NT : 2 * NT],
                                         lhsT=bsel[:, 128 * g : 128 * g + 128],
                                         rhs=wg[:, off : off + NT],
                                         start=True, stop=False)
                        for pi, part in enumerate(("mu", "sig")):
                            po = NT * pi
                            for k in range(2):
                                for j in range(4):
                                    nc.tensor.matmul(
                                        pg[32 * j : 32 * j + 32, po : po + NT],
                                        lhsT=BW(part, g, k)[:, 32 * j : 32 * j + 32],
                                        rhs=h[4 * g + j][k][:, off : off + NT],
                                        start=(k == 0 and pi == 0),
                                        stop=(k == 1),
                                        tile_position=(0, 32 * j),
                                    )
                        # inv = exp(glogit - (Wsig h + bsig)) directly
                        iv = mix4.tile([128, NT], BF16, tag="inv")
                        nc.scalar.activation(out=iv, in_=pg[:, NT : 2 * NT],
                                             func=EX,
                                             bias=bias(f"nbsig_{g}"), scale=-1.0)
                        mv = mix4.tile([128, NT], BF16, tag="minv")
                        nc.vector.tensor_mul(mv, pg[:, 0:NT], iv)
                        inv.append(iv)
                        minv.append(mv)
                    halves.append((inv, minv))
                return dict(b0=st["b0"], aT=st["aT"], halves=halves)

            def stage4(st):
                b0 = st["b0"]
                stack = smC.tile([64, MT], BF16, tag="stack")
                for hh in range(2):
                    off = NT * hh
                    inv, minv = st["halves"][hh]
                    ps1 = psS.tile([32, NT], F32, tag="small")
                    nc.tensor.matmul(ps1, lhsT=W("Ssel", 0, 32), rhs=minv[0],
                                     start=True, stop=False)
                    nc.tensor.matmul(ps1, lhsT=W("Ssel", 0, 32), rhs=minv[1],
                                     start=False, stop=False)
                    nc.tensor.matmul(ps1, lhsT=W("SselB0", 0, 32), rhs=inv[0],
                                     start=False, stop=False)
                    nc.tensor.matmul(ps1, lhsT=W("SselB1", 0, 32), rhs=inv[1],
                                     start=False, stop=True)
                    ps2 = psS.tile([32, NT], F32, tag="small")
                    nc.tensor.matmul(ps2, lhsT=W("Ssel", 0, 32), rhs=inv[0],
                                     start=True, stop=False)
                    nc.tensor.matmul(ps2, lhsT=W("Ssel", 0, 32), rhs=inv[1],
                                     start=False, stop=True)

                    za = smC.tile([32, NT], F32, tag="za")
                    nc.vector.tensor_mul(za, st["aT"][:, off : off + NT], ps2)
                    zb = smC.tile([32, NT], F32, tag="zb")
                    nc.vector.tensor_sub(zb, za, ps1)
                    nc.vector.tensor_mul(stack[0:32, off : off + NT], zb, zb)
                    nc.scalar.activation(out=stack[32:64, off : off + NT],
                                         in_=ps2, func=LN,
                                         bias=biasn("lnbias", 32), scale=1.0)
                return dict(b0=b0, stack=stack)

            def stage5(st):
                b0, stack = st["b0"], st["stack"]
                outs = smC.tile([2, MT], F32, tag="outs")
                for hh in range(2):
                    off = NT * hh
                    pso = psS.tile([2, NT], F32, tag="small")
                    nc.tensor.matmul(pso, lhsT=finw[0:64, :],
                                     rhs=stack[:, off : off + NT],
                                     start=True, stop=True)
                    nc.vector.tensor_copy(outs[:, off : off + NT], pso)
                nc.sync.dma_start(out=out_t[:, b0 : b0 + MT], in_=outs)

            live = {}
            for t in range(NMAC + 5):
                # deepest stage first: frees pool slots before new allocs
                if 0 <= t - 5 < NMAC:
                    stage5(live.pop((t - 5, 5)))
                if 0 <= t - 4 < NMAC:
                    live[(t - 4, 5)] = stage4(live.pop((t - 4, 4)))
                if 0 <= t - 3 < NMAC:
                    live[(t - 3, 4)] = stage3(live.pop((t - 3, 3)))
                if 0 <= t - 2 < NMAC:
                    live[(t - 2, 3)] = stage2(live.pop((t - 2, 2)))
                if 0 <= t - 1 < NMAC:
                    live[(t - 1, 2)] = stage1(live.pop((t - 1, 1)))
                if t < NMAC:
                    live[(t, 1)] = stage0(t)

    nc.compile()
    return nc


_dummy_packs = None


def _build_packs_dummy():
    """Offsets only (shapes fixed) — computed once with zero weights."""
    global _dummy_packs
    if _dummy_packs is None:
        zi = {
            "W1": np.zeros((256, 512), np.float32),
            "W2": np.zeros((512, 256), np.float32),
            "Wp1": np.zeros((8, 256, 256), np.float32),
            "Wp2": np.zeros((8, 256, 40), np.float32),
            "Ws1": np.zeros((256, 512), np.float32),
            "Ws2": np.zeros((512, 256), np.float32),
            "Ws3": np.zeros((256, 128), np.float32),
            "Wg1": np.zeros((128, 512), np.float32),
            "Wg2": np.zeros((512, 256), np.float32),
            "Wg3": np.zeros((256, 128), np.float32),
            "Wgate": np.zeros((256, 8), np.float32),
            "b1": np.zeros(512, np.float32),
            "b2": np.zeros(256, np.float32),
            "bp1": np.zeros((8, 256), np.float32),
            "bp2": np.zeros((8, 40), np.float32),
            "bs1": np.zeros(512, np.float32),
            "bs2": np.zeros(256, np.float32),
            "bs3": np.zeros(128, np.float32),
            "bg1": np.zeros(512, np.float32),
            "bg2": np.zeros(256, np.float32),
            "bg3": np.zeros(128, np.float32),
            "bgate": np.zeros(8, np.float32),
        }
        _dummy_packs = _build_packs(zi)
    return _dummy_packs


_nc_cache = None


def _get_nc():
    global _nc_cache
    if _nc_cache is None:
        _nc_cache = _build_nc()
    return _nc_cache


def run(inputs, trace=False):
    """Returns (out [B,2] fp32, exec_time_ns or None)."""
    inp = {k: np.asarray(v) for k, v in inputs.items()}
    wpk, _, bpk, _, bia, _, bsel, finw = _build_packs(inp)

    obs = inp["obs"].astype(np.float32)
    act = inp["actions"].astype(np.float32)

    in_maps = []
    for c in range(N_CORES):
        sl = slice(c * BC, (c + 1) * BC)
        obs_tr = np.ascontiguousarray(obs[sl].T.astype(ml_dtypes.bfloat16))
        a32 = np.zeros((32, BC), np.float32)
        a32[:A] = act[sl].T
        in_maps.append({
            "obs_t": obs_tr, "act_t": a32, "wpk": wpk,
            "bpk": np.ascontiguousarray(bpk), "bia": bia,
            "bsel": bsel, "finw": finw,
        })

    nc = _get_nc()
    res = run_bass_kernel_spmd(nc, in_maps, core_ids=list(range(N_CORES)),
                               trace=trace)
    parts = []
    for c in range(N_CORES):
        ot = res.results[c]["out_t"]  # [2, BC]
        parts.append(ot.T)
    out = np.concatenate(parts, axis=0).astype(np.float32)
    return out, res.exec_time_ns


def kernel(**inputs) -> np.ndarray:
    out, _ = run(inputs, trace=False)
    return out


# revision 21
# speedup vs baseline: 1.0114x; 1.0114x over previous
"""TRN2 Bass kernel for nn_Actor (moe_routing): gated mixture actor forward.

Strategy: pure data parallel over 8 NeuronCores (batch 65536 -> 8192/core),
weights replicated. On-device layout keeps features in partitions and batch
in the free dimension, so activations stream through the TensorEngine as the
moving operand and weights stay stationary. The host pre-transposes obs and
actions, so the device does zero transposes.

Matmuls run in bf16 (1 cycle/row; fp32 is 4, float32r 1.5) accumulating in
fp32 PSUM. Elementwise ops run on [128, 1024] macro-tiles (two 512-wide
matmul halves per PSUM pair-tile) to amortize the ACT engine's 352-cycle
per-op overhead; leaky-relu work is split across ACT (fused Prelu+bias from
PSUM) and DVE (3-op chain). The per-primitive output heads (20 mu + 20
sigma cols each) are packed four-to-a-PSUM-tile at 32-partition col
positions (tile_position col packing). Biases ride the ACT ops; the mu-head
bias is folded into the primitive reduction as a weighted selection matrix,
and the gate weighting is folded into the sigma exponent
(w*exp(-s) = exp(glogit - s)) via a negated broadcast matmul accumulated
into the same PSUM tile. log_prob/entropy come from one K=64 reduction
matmul whose constant term rides a ln(e)=1 row.

The work is a 6-stage software pipeline across macro-tiles (DMA; L1; L2/L3/
gate-head/towers; tower-out+mixture; reduction+z/ln; final matmul+store),
emitted deepest-stage-first, so every TensorEngine instruction's inputs were
produced at least one full macro-tile earlier and the PE never stalls on the
ACT/DVE chains. All activation functions (Prelu/Exp/Ln) come from one ACT
table set (natural_log_exp_and_others, forced via a table-selection patch)
so there are zero mid-kernel table reloads.
"""

import numpy as np
import ml_dtypes

import concourse.bass as bass
import concourse.tile as tile
from concourse import bacc, mybir
from concourse.bass_utils import run_bass_kernel_spmd

F32 = mybir.dt.float32
F32R = mybir.dt.float32r
BF16 = mybir.dt.bfloat16

B = 65536
N_CORES = 8
BC = B // N_CORES          # rows per core
NT = 512                   # matmul moving width / PSUM bank width
MT = 1024                  # macro-tile batch width (elementwise op width)
NMAC = BC // MT            # 8
NTILES = BC // NT          # 16
A = 20                     # action dim
P = 8                      # primitives
C = 0.5 * float(np.log(2.0 * np.pi))  # LOG_SQRT_2PI
ALPHA = 0.01

# number of tower-h chunks (out of 16) whose leaky relu runs on DVE not ACT
N_H_DVE = 6

_WPK_COLS = 10352
_BIA_COLS = 41


def _force_single_act_table():
    """Strip Exp/Ln/Prelu from every activation-function set except
    natural_log_exp_and_others, so bacc's table-load placement resolves all
    three to one set and hoists a single load (instead of ping-ponging
    between sets every tile, 1.3us per switch)."""
    import concourse.hw_specs as hw_specs
    import concourse.bacc as bacc_mod

    orig = hw_specs.get_activation_tables
    KEEP = "natural_log_exp_and_others"
    STRIP = {
        mybir.ActivationFunctionType.Exp,
        mybir.ActivationFunctionType.Ln,
        mybir.ActivationFunctionType.Prelu,
    }

    def patched(arch):
        tabs = dict(orig(arch))
        out = {}
        for name, fns in tabs.items():
            if name == KEEP:
                out[name] = fns
            else:
                out[name] = {f for f in fns if f not in STRIP}
        return out

    bacc_mod.get_activation_tables = patched


# ---------------------------------------------------------------- host packs
def _chunk_cols(w):
    """[K, M] -> [128, (K/128)*M] chunked lhsT layout."""
    K, M = w.shape
    nk = K // 128
    out = np.empty((128, nk * M), dtype=np.float32)
    for k in range(nk):
        out[:, k * M : (k + 1) * M] = w[k * 128 : (k + 1) * 128]
    return out


def _build_packs(inp):
    wcols = []
    woff = {}

    def addw(name, arr):
        woff[name] = sum(a.shape[1] for a in wcols)
        wcols.append(arr.astype(np.float32))

    addw("W1", _chunk_cols(inp["W1"]))
    addw("Ws1", _chunk_cols(inp["Ws1"]))
    addw("Wg1", _chunk_cols(inp["Wg1"]))
    addw("W2", _chunk_cols(inp["W2"]))
    for p in range(P):
        addw(f"Wp1_{p}", _chunk_cols(inp["Wp1"][p]))
    addw("Ws2", _chunk_cols(inp["Ws2"]))
    addw("Ws3", _chunk_cols(inp["Ws3"]))
    addw("Wg2", _chunk_cols(inp["Wg2"]))
    addw("Wg3", _chunk_cols(inp["Wg3"]))
    addw("Wgate", _chunk_cols(inp["Wgate"]))
    ssel = np.zeros((128, 32), np.float32)
    for j in range(4):
        for a in range(A):
            ssel[32 * j + a, a] = 1.0
    addw("Ssel", ssel)
    # weighted selection: adds sum_p bmu_p * inv_p to sum1 (mu-bias term)
    for g in range(2):
        sb = np.zeros((128, 32), np.float32)
        for j in range(4):
            sb[32 * j : 32 * j + A, :A] = np.diag(inp["bp2"][4 * g + j][:A])
        addw(f"SselB{g}", sb)
    wpk = np.concatenate(wcols, axis=1).astype(ml_dtypes.bfloat16)
    assert wpk.shape[1] == _WPK_COLS, wpk.shape
    finw = np.zeros((128, 2), np.float32)
    finw[0:A, 0] = -0.5
    finw[32 : 32 + A, 0] = 1.0
    finw[32 : 32 + A, 1] = -1.0
    finw[63, 0] = -A * C
    finw[63, 1] = A * (0.5 + C)
    finw = finw.astype(ml_dtypes.bfloat16)

    # bf16 tower-out weights: [128, 8 * 128], order (pass mu/sig, group, kchunk)
    bpk = np.zeros((128, 8, 128), np.float32)
    bi = 0
    boff = {}
    for part, off in (("mu", 0), ("sig", A)):
        for g in range(2):
            for k in range(2):
                boff[(part, g, k)] = bi
                for j in range(4):
                    w = inp["Wp2"][4 * g + j][128 * k : 128 * k + 128, off : off + A]
                    bpk[:, bi, 32 * j : 32 * j + A] = w
                bi += 1
    bpk = bpk.reshape(128, 8 * 128).astype(ml_dtypes.bfloat16)

    # ACT bias pack [128, CA] fp32
    bcols = []
    aoff = {}

    def addb(name, vec):
        aoff[name] = len(bcols)
        col = np.zeros(128, np.float32)
        col[: len(vec)] = vec
        bcols.append(col)

    for m in range(4):
        addb(f"b1_{m}", inp["b1"][128 * m : 128 * m + 128])
    for m in range(2):
        addb(f"b2_{m}", inp["b2"][128 * m : 128 * m + 128])
    for p in range(P):
        for m in range(2):
            addb(f"bp1_{p}_{m}", inp["bp1"][p][128 * m : 128 * m + 128])
    for m in range(4):
        addb(f"bs1_{m}", inp["bs1"][128 * m : 128 * m + 128])
    for m in range(2):
        addb(f"bs2_{m}", inp["bs2"][128 * m : 128 * m + 128])
    addb("bs3_0", inp["bs3"])
    for m in range(4):
        addb(f"bg1_{m}", inp["bg1"][128 * m : 128 * m + 128])
    for m in range(2):
        addb(f"bg2_{m}", inp["bg2"][128 * m : 128 * m + 128])
    addb("bg3_0", inp["bg3"])
    addb("bgate", inp["bgate"])
    lnb = np.zeros(128, np.float32)
    lnb[A:32] = 1.0
    lnb[31] = float(np.e)  # stack row 63 becomes ln(e)=1 -> constant term
    addb("lnbias", lnb)
    addb("zero", np.zeros(1, np.float32))
    for g in range(2):  # negated sigma-logit bias, group layout
        nb = np.zeros(128, np.float32)
        for j in range(4):
            nb[32 * j : 32 * j + A] = -inp["bp2"][4 * g + j][A:]
        addb(f"nbsig_{g}", nb)
    bia = np.stack(bcols, axis=1)
    assert bia.shape[1] == _BIA_COLS, bia.shape

    bsel = np.zeros((8, 256), np.float32)
    for g in range(2):
        for j in range(4):
            bsel[4 * g + j, 128 * g + 32 * j : 128 * g + 32 * j + A] = -1.0
    bsel = bsel.astype(ml_dtypes.bfloat16)

    return wpk, woff, bpk, boff, bia, aoff, bsel, finw


# ---------------------------------------------------------------- bass build
def _build_nc():
    _force_single_act_table()
    nc = bacc.Bacc("TRN2", target_bir_lowering=False, debug=False,
                   num_devices=N_CORES)

    obs_t = nc.dram_tensor("obs_t", [384, BC], BF16, kind="ExternalInput").ap()
    act_t = nc.dram_tensor("act_t", [32, BC], F32, kind="ExternalInput").ap()
    wpk_d = nc.dram_tensor("wpk", [128, _WPK_COLS], BF16, kind="ExternalInput").ap()
    bpk_d = nc.dram_tensor("bpk", [128, 1024], BF16, kind="ExternalInput").ap()
    bia_d = nc.dram_tensor("bia", [128, _BIA_COLS], F32, kind="ExternalInput").ap()
    bsel_d = nc.dram_tensor("bsel", [8, 256], BF16, kind="ExternalInput").ap()
    finw_d = nc.dram_tensor("finw", [128, 2], BF16, kind="ExternalInput").ap()
    out_t = nc.dram_tensor("out_t", [2, BC], F32, kind="ExternalOutput").ap()

    _, woff, _, boff, _, aoff, _, _ = _build_packs_dummy()

    PR = mybir.ActivationFunctionType.Prelu
    EX = mybir.ActivationFunctionType.Exp
    LN = mybir.ActivationFunctionType.Ln

    with tile.TileContext(nc) as tc:
        with (
            tc.tile_pool(name="wgt", bufs=1) as wgt,
            tc.tile_pool(name="obs", bufs=2) as obsp,
            tc.tile_pool(name="quad", bufs=4) as quad,
            tc.tile_pool(name="dual", bufs=3) as dual,
            tc.tile_pool(name="uni", bufs=3) as uni,
            tc.tile_pool(name="mix4", bufs=6) as mix4,
            tc.tile_pool(name="lr", bufs=2) as lrp,
            tc.tile_pool(name="hp", bufs=18) as hp,
            tc.tile_pool(name="smA", bufs=5) as smA,
            tc.tile_pool(name="smB", bufs=3) as smB,
            tc.tile_pool(name="smC", bufs=3) as smC,
            tc.tile_pool(name="psA", bufs=3, space="PSUM") as psA,
            tc.tile_pool(name="psS", bufs=2, space="PSUM") as psS,
        ):
            SPLIT = 2560  # W1 + Ws1 + Wg1: needed by the first L1 matmuls
            wpkA = wgt.tile([128, SPLIT], BF16)
            wpkB = wgt.tile([128, _WPK_COLS - SPLIT], BF16)
            bpk = wgt.tile([128, 1024], BF16)
            bia = wgt.tile([128, _BIA_COLS], F32)
            bsel = wgt.tile([8, 256], BF16)
            finw = wgt.tile([128, 2], BF16)
            nc.sync.dma_start(out=bia, in_=bia_d)
            nc.sync.dma_start(out=wpkA, in_=wpk_d[:, 0:SPLIT])
            nc.sync.dma_start(out=wpkB, in_=wpk_d[:, SPLIT:_WPK_COLS])
            nc.sync.dma_start(out=bpk, in_=bpk_d)
            nc.sync.dma_start(out=bsel, in_=bsel_d)
            nc.sync.dma_start(out=finw, in_=finw_d)

            def W(name, k, M):
                off = woff[name] + k * M
                if off < SPLIT:
                    return wpkA[:, off : off + M]
                return wpkB[:, off - SPLIT : off - SPLIT + M]

            def BW(part, g, k):
                i = boff[(part, g, k)]
                return bpk[:, 128 * i : 128 * i + 128]

            def bias(name):
                return bia[:, aoff[name] : aoff[name] + 1]

            def biasn(name, n):
                return bia[0:n, aoff[name] : aoff[name] + 1]

            def layer(x_chunks, wname, bname, n_in, n_out, pool, out_tag,
                      eng="act", out_dt=BF16):
                engs = eng if isinstance(eng, list) else [eng] * n_out
                outs = []
                for m in range(n_out):
                    eng = engs[m]
                    ps = psA.tile([128, MT], F32, tag="mm")
                    for k in range(n_in):
                        wk = W(wname, k, 128 * n_out)[:, 128 * m : 128 * m + 128]
                        for hh in range(2):
                            nc.tensor.matmul(
                                ps[:, NT * hh : NT * hh + NT], lhsT=wk,
                                rhs=x_chunks[k][:, NT * hh : NT * hh + NT],
                                start=(k == 0), stop=(k == n_in - 1),
                            )
                    o = pool.tile([128, MT], out_dt, tag=out_tag)
                    b = bias(f"{bname}_{m}")
                    if eng == "dve":
                        t = lrp.tile([128, MT], BF16, tag="lrt")
                        u = lrp.tile([128, MT], BF16, tag="lru")
                        nc.vector.tensor_scalar_add(t, ps, b)
                        nc.vector.tensor_scalar_mul(u, t, ALPHA)
                        nc.vector.tensor_max(o, t, u)
                    else:
                        nc.scalar.activation(out=o, in_=ps, func=PR, bias=b,
                                             scale=1.0, alpha=ALPHA)
                    outs.append(o)
                return outs

            def stage0(mt):
                b0 = mt * MT
                obs0 = obsp.tile([128, MT], BF16, tag="o0")
                obs1 = obsp.tile([128, MT], BF16, tag="o1")
                obs2 = obsp.tile([128, MT], BF16, tag="o2")
                nc.sync.dma_start(out=obs0, in_=obs_t[0:128, b0 : b0 + MT])
                nc.sync.dma_start(out=obs1, in_=obs_t[128:256, b0 : b0 + MT])
                nc.sync.dma_start(out=obs2, in_=obs_t[256:384, b0 : b0 + MT])
                aT = smA.tile([32, MT], F32, tag="aT")
                nc.sync.dma_start(out=aT, in_=act_t[:, b0 : b0 + MT])
                return dict(b0=b0, aT=aT, obs=[obs0, obs1, obs2])

            def stage1(st):
                obs0, obs1, obs2 = st["obs"]
                t1 = layer([obs0, obs1], "Ws1", "bs1", 2, 4, quad, "gs1",
                           eng=["act", "dve", "act", "dve"])
                v1 = layer([obs2], "Wg1", "bg1", 1, 4, quad, "gg1",
                           eng=["act", "dve", "act", "act"])
                s1 = layer([obs0, obs1], "W1", "b1", 2, 4, quad, "s1",
                           eng=["act", "dve", "act", "dve"])
                return dict(b0=st["b0"], aT=st["aT"], t1=t1, v1=v1, s1=s1)

            def stage2(st):
                t2 = layer(st["t1"], "Ws2", "bs2", 4, 2, dual, "gs2")
                v2 = layer(st["v1"], "Wg2", "bg2", 4, 2, dual, "gg2")
                s1b = layer(st["s1"], "W2", "b2", 4, 2, dual, "s1b")
                s2 = layer(t2, "Ws3", "bs3", 2, 1, uni, "gs3")[0]
                g2 = layer(v2, "Wg3", "bg3", 2, 1, uni, "gg3")[0]
                wg = smB.tile([8, MT], BF16, tag="wg")
                for hh in range(2):
                    off = NT * hh
                    psg = psS.tile([8, NT], F32, tag="small")
                    nc.tensor.matmul(psg, lhsT=W("Wgate", 0, 8),
                                     rhs=s2[:, off : off + NT],
                                     start=True, stop=False)
                    nc.tensor.matmul(psg, lhsT=W("Wgate", 1, 8),
                                     rhs=g2[:, off : off + NT],
                                     start=False, stop=True)
                    nc.scalar.activation(out=wg[:, off : off + NT], in_=psg,
                                         func=mybir.ActivationFunctionType.Identity,
                                         bias=biasn("bgate", 8), scale=1.0)
                ENG = ["dve", "act", "dve", "act", "dve", "act", "act", "act"]
                h = []
                for p in range(P):
                    h.append(layer(s1b, f"Wp1_{p}", f"bp1_{p}", 2, 2, hp,
                                   "h", eng=ENG[p], out_dt=BF16))
                return dict(b0=st["b0"], aT=st["aT"], wg=wg, h=h)

            def stage3(st):
                h, wg = st["h"], st["wg"]
                halves = []
                for hh in range(2):
                    off = NT * hh
                    inv, minv = [], []
                    for g in range(2):
                        pg = psA.tile([128, MT], F32, tag="mm")
                        # -glogit broadcast seeds the sig half (start=True)
                        nc.tensor.matmul(pg[:, NT : 2 * NT],
                                         lhsT=bsel[:, 128 * g : 128 * g + 128],
                                         rhs=wg[:, off : off + NT],
                                         start=True, stop=False)
                        for pi, part in enumerate(("mu", "sig")):
                            po = NT * pi
                            for k in range(2):
                                for j in range(4):
                                    nc.tensor.matmul(
                                        pg[32 * j : 32 * j + 32, po : po + NT],
                                        lhsT=BW(part, g, k)[:, 32 * j : 32 * j + 32],
                                        rhs=h[4 * g + j][k][:, off : off + NT],
                                        start=(k == 0 and pi == 0),
                                        stop=(k == 1),
                                        tile_position=(0, 32 * j),
                                    )
                        # inv = exp(glogit - (Wsig h + bsig)) directly
                        iv = mix4.tile([128, NT], BF16, tag="inv")
                        nc.scalar.activation(out=iv, in_=pg[:, NT : 2 * NT],
                                             func=EX,
                                             bias=bias(f"nbsig_{g}"), scale=-1.0)
                        mv = mix4.tile([128, NT], BF16, tag="minv")
                        nc.vector.tensor_mul(mv, pg[:, 0:NT], iv)
                        inv.append(iv)
                        minv.append(mv)
                    halves.append((inv, minv))
                return dict(b0=st["b0"], aT=st["aT"], halves=halves)

            def stage4(st):
                b0 = st["b0"]
                stack = smC.tile([64, MT], BF16, tag="stack")
                for hh in range(2):
                    off = NT * hh
                    inv, minv = st["halves"][hh]
                    ps1 = psS.tile([32, NT], F32, tag="small")
                    nc.tensor.matmul(ps1, lhsT=W("Ssel", 0, 32), rhs=minv[0],
                                     start=True, stop=False)
                    nc.tensor.matmul(ps1, lhsT=W("Ssel", 0, 32), rhs=minv[1],
                                     start=False, stop=False)
                    nc.tensor.matmul(ps1, lhsT=W("SselB0", 0, 32), rhs=inv[0],
                                     start=False, stop=False)
                    nc.tensor.matmul(ps1, lhsT=W("SselB1", 0, 32), rhs=inv[1],
                                     start=False, stop=True)
                    ps2 = psS.tile([32, NT], F32, tag="small")
                    nc.tensor.matmul(ps2, lhsT=W("Ssel", 0, 32), rhs=inv[0],
                                     start=True, stop=False)
                    nc.tensor.matmul(ps2, lhsT=W("Ssel", 0, 32), rhs=inv[1],
                                     start=False, stop=True)

                    za = smC.tile([32, NT], F32, tag="za")
                    nc.vector.tensor_mul(za, st["aT"][:, off : off + NT], ps2)
                    zb = smC.tile([32, NT], F32, tag="zb")
                    nc.vector.tensor_sub(zb, za, ps1)
                    nc.vector.tensor_mul(stack[0:32, off : off + NT], zb, zb)
                    nc.scalar.activation(out=stack[32:64, off : off + NT],
                                         in_=ps2, func=LN,
                                         bias=biasn("lnbias", 32), scale=1.0)
                return dict(b0=b0, stack=stack)

            def stage5(st):
                b0, stack = st["b0"], st["stack"]
                outs = smC.tile([2, MT], F32, tag="outs")
                for hh in range(2):
                    off = NT * hh
                    pso = psS.tile([2, NT], F32, tag="small")
                    nc.tensor.matmul(pso, lhsT=finw[0:64, :],
                                     rhs=stack[:, off : off + NT],
                                     start=True, stop=True)
                    nc.vector.tensor_copy(outs[:, off : off + NT], pso)
                nc.sync.dma_start(out=out_t[:, b0 : b0 + MT], in_=outs)

            live = {}
            for t in range(NMAC + 5):
                # deepest stage first: frees pool slots before new allocs
                if 0 <= t - 5 < NMAC:
                    stage5(live.pop((t - 5, 5)))
                if 0 <= t - 4 < NMAC:
                    live[(t - 4, 5)] = stage4(live.pop((t - 4, 4)))
                if 0 <= t - 3 < NMAC:
                    live[(t - 3, 4)] = stage3(live.pop((t - 3, 3)))
                if 0 <= t - 2 < NMAC:
                    live[(t - 2, 3)] = stage2(live.pop((t - 2, 2)))
                if 0 <= t - 1 < NMAC:
                    live[(t - 1, 2)] = stage1(live.pop((t - 1, 1)))
                if t < NMAC:
                    live[(t, 1)] = stage0(t)

    nc.compile()
    return nc


_dummy_packs = None


def _build_packs_dummy():
    """Offsets only (shapes fixed) — computed once with zero weights."""
    global _dummy_packs
    if _dummy_packs is None:
        zi = {
            "W1": np.zeros((256, 512), np.float32),
            "W2": np.zeros((512, 256), np.float32),
            "Wp1": np.zeros((8, 256, 256), np.float32),
            "Wp2": np.zeros((8, 256, 40), np.float32),
            "Ws1": np.zeros((256, 512), np.float32),
            "Ws2": np.zeros((512, 256), np.float32),
            "Ws3": np.zeros((256, 128), np.float32),
            "Wg1": np.zeros((128, 512), np.float32),
            "Wg2": np.zeros((512, 256), np.float32),
            "Wg3": np.zeros((256, 128), np.float32),
            "Wgate": np.zeros((256, 8), np.float32),
            "b1": np.zeros(512, np.float32),
            "b2": np.zeros(256, np.float32),
            "bp1": np.zeros((8, 256), np.float32),
            "bp2": np.zeros((8, 40), np.float32),
            "bs1": np.zeros(512, np.float32),
            "bs2": np.zeros(256, np.float32),
            "bs3": np.zeros(128, np.float32),
            "bg1": np.zeros(512, np.float32),
            "bg2": np.zeros(256, np.float32),
            "bg3": np.zeros(128, np.float32),
            "bgate": np.zeros(8, np.float32),
        }
        _dummy_packs = _build_packs(zi)
    return _dummy_packs


_nc_cache = None


def _get_nc():
    global _nc_cache
    if _nc_cache is None:
        _nc_cache = _build_nc()
    return _nc_cache


def run(inputs, trace=False):
    """Returns (out [B,2] fp32, exec_time_ns or None)."""
    inp = {k: np.asarray(v) for k, v in inputs.items()}
    wpk, _, bpk, _, bia, _, bsel, finw = _build_packs(inp)

    obs = inp["obs"].astype(np.float32)
    act = inp["actions"].astype(np.float32)

    in_maps = []
    for c in range(N_CORES):
        sl = slice(c * BC, (c + 1) * BC)
        obs_tr = np.ascontiguousarray(obs[sl].T.astype(ml_dtypes.bfloat16))
        a32 = np.zeros((32, BC), np.float32)
        a32[:A] = act[sl].T
        in_maps.append({
            "obs_t": obs_tr, "act_t": a32, "wpk": wpk,
            "bpk": np.ascontiguousarray(bpk), "bia": bia,
            "bsel": bsel, "finw": finw,
        })

    nc = _get_nc()
    res = run_bass_kernel_spmd(nc, in_maps, core_ids=list(range(N_CORES)),
                               trace=trace)
    parts = []
    for c in range(N_CORES):
        ot = res.results[c]["out_t"]  # [2, BC]
        parts.append(ot.T)
    out = np.concatenate(parts, axis=0).astype(np.float32)
    return out, res.exec_time_ns


def kernel(**inputs) -> np.ndarray:
    out, _ = run(inputs, trace=False)
    return out
